# revision 5
# baseline (speedup 1.0000x reference)
"""Trainium2 Bass kernel for batched pairwise-distance + group-min + mean.

Computes, for x1 [8, 2048, 1024] f32 and x2 [8, 1152, 1024] f32:
    d[b, m, n] = ||x1[b,m] - x2[b,n]||^2           [8, 2048, 1152]
    out = mean over groups-of-9 minima of d (reshape [B, -1, 9].min(-1).mean())

Strategy: data-parallel over batch B=8 across the 8 NeuronCores. Each core:
  - cast-DMAs x1/x2 to fp8e4 (e4m3) in SBUF,
  - transposes them with the xbar DMA by viewing adjacent fp8 pairs as one
    bf16 element (halves transpose traffic); the pair-interleaved layout is
    consumed by DoubleRowSwInterleave fp8 matmuls (the reversed row order
    they imply only permutes PSUM partitions, which the final sum ignores),
  - computes cross[m, n] = x1 @ x2.T in fp8 at 2 k-rows/partition/cycle,
  - appends -0.5*||x2[n]||^2 per column with a K=1 bf16 ones matmul
    (sq2 via ACT square-accumulate, PE-transposed + DMA-flattened to a row),
  - group-MAX of (cross - 0.5*sq2) over 9 consecutive n on the vector engine
    (min of d is sq1[m] - 2 * that max; sq1 is constant within a group),
  - sq1 via ACT square-accumulate; host combines:
        sum_d_min = 128 * sum(sq1) - 2 * sum(group_max_sums)
"""
import os
import sys

for _p in ("/opt/trn_rl_repo",):
    if os.path.isdir(_p) and _p not in sys.path:
        sys.path.append(_p)

import numpy as np

B = 8
N1, D, N2 = 2048, 1024, 1152
GROUP = 9
MT = N1 // 128                        # 16 m-tiles
KK = 4                                 # DoubleRow k-steps (4 x 256 = 1024)
TLX = N2 // 128                        # 9 x2 column tiles
NG = N2 // GROUP                       # 128 groups per m-row
# psum free-dim chunks for the 256-col fp8 mains (bank-aligned)
MCHUNK = ((0, 256), (256, 256), (512, 256), (768, 256), (1024, 128))
ACHUNK = ((0, 512), (512, 512), (1024, 128))

_CACHE = {}


def _build():
    """Build + compile the per-core Bass program once per process."""
    from concourse import bacc, tile, mybir
    from concourse.masks import make_identity

    F32 = mybir.dt.float32
    BF = mybir.dt.bfloat16
    F8 = mybir.dt.float8e4
    AX = mybir.AxisListType
    AF = mybir.ActivationFunctionType
    DR = mybir.MatmulPerfMode.DoubleRowSwInterleave

    nc = bacc.Bacc("TRN2", target_bir_lowering=False, debug=False, num_devices=B,
                   dynamic_dma_scratch_size=65536)
    x1_d = nc.dram_tensor("x1", [N1, D], F32, kind="ExternalInput")
    x2_d = nc.dram_tensor("x2", [N2, D], F32, kind="ExternalInput")
    y_gm = nc.dram_tensor("y_gm", [128, MT], F32, kind="ExternalOutput")
    y_sq1 = nc.dram_tensor("y_sq1", [128, MT], F32, kind="ExternalOutput")

    with tile.TileContext(nc) as tc:
        with tc.tile_pool(name="big", bufs=1) as big, \
             tc.tile_pool(name="work", bufs=2) as workp, \
             tc.tile_pool(name="ps", bufs=2, space="PSUM") as psp, \
             tc.tile_pool(name="psaux", bufs=1, space="PSUM") as psaux:

            x1p8 = big.tile([128, MT, D], F8)       # x1, m-major, fp8
            x2p8 = big.tile([128, TLX, D], F8)      # x2, n-major, fp8
            X1T = big.tile([128, KK, N1], BF)       # pair-transposed, kk-major
            X2T = big.tile([128, KK, N2], BF)
            ident = big.tile([128, 128], BF)
            ones_bf = big.tile([1, 128], BF)
            sq2c = big.tile([128, TLX], F32)
            sq2cb = big.tile([128, TLX], BF)
            sq2T = big.tile([TLX, 128], BF)
            row0 = big.tile([1, N2], BF)
            sq1_out = big.tile([128, MT], F32)
            gm_all = big.tile([128, MT, NG], BF)
            y_gm_t = big.tile([128, MT], F32)
            warm_src = big.tile([128, 512], BF)

            # ---- loads first (SWDGE, f32 -> fp8 cast): x1 slice-group 0,
            #      then all of x2, then the rest of x1 ----
            x1_view = x1_d.ap().rearrange("(g s p) d -> g p s d", g=4, p=128)
            nc.gpsimd.dma_start(out=x1p8[:, 0:4, :], in_=x1_view[0])
            x2_view = x2_d.ap().rearrange("(g s p) d -> g p s d", g=3, p=128)
            for g in range(3):
                nc.gpsimd.dma_start(out=x2p8[:, 3 * g:3 * g + 3, :], in_=x2_view[g])
            for g in range(1, 4):
                nc.gpsimd.dma_start(out=x1p8[:, 4 * g:4 * g + 4, :], in_=x1_view[g])

            # ---- constants (cheap, after load dispatch) ----
            make_identity(nc, ident)
            nc.vector.memset(ones_bf[:], 1.0)
            nc.gpsimd.memset(warm_src[:], 0.0)

            # ---- transposes (HWDGE xbar on bf16 pair view), eager; x1
            #      slices 0-1 first so the PE can start, then x2, then the
            #      remaining x1 slices ----
            def x1_transpose(s):
                eng = nc.sync if s % 2 == 0 else nc.scalar
                eng.dma_start(out=X1T[:, :, s * 128:(s + 1) * 128],
                              in_=x1p8[:, s, :].bitcast(BF), transpose=True)

            def x2_transpose(s):
                nc.sync.dma_start(out=X2T[:, :, s * 128:(s + 1) * 128],
                                  in_=x2p8[:, s, :].bitcast(BF), transpose=True)

            for s in range(2):
                x1_transpose(s)
            for s in range(TLX):
                x2_transpose(s)
                # sq2 squares on ACT, one per slice (row0 latency critical)
                nc.scalar.activation(out=workp.tile([128, D], BF, tag="scr",
                                                    name=f"scr2_{s}")[:],
                                     in_=x2p8[:, s, :], func=AF.Square,
                                     accum_out=sq2c[:, s:s + 1])
            for s in range(2, MT):
                x1_transpose(s)

            # ---- sq2 row: scale -0.5, PE-transpose to [9,128], flatten to
            #      [1,1152] with a tiny 9-descriptor DMA ----
            nc.vector.tensor_scalar_mul(sq2cb[:], sq2c[:], -0.5)
            ps_t = psaux.tile([TLX, 128], BF, tag="tr", name="sq2T_ps")
            nc.tensor.matmul(ps_t[:], lhsT=sq2cb[:], rhs=ident[:],
                             is_transpose=True)
            nc.vector.tensor_copy(sq2T[:], ps_t[:])
            nc.gpsimd.dma_start(
                out=row0[:].rearrange("o (s n) -> o s n", n=128), in_=sq2T[:])

            # ---- PE warmup: junk matmuls ramp the pstate clock before the
            #      real mains issue (they only touch a scratch bank) ----
            ps_w = psaux.tile([128, 512], F32, tag="warm", name="warm")
            for i in range(8):
                nc.tensor.matmul(ps_w[:], lhsT=ident[:], rhs=warm_src[:],
                                 start=True, stop=True, skip_group_check=True)

            # ---- main loop: fp8 SwInterleave mains (256-col chunks) + bf16
            #      ones-row append, then DVE group-max epilogue ----
            X1T8 = X1T[:].bitcast(F8)               # [128, KK, 2*N1]
            X2T8 = X2T[:].bitcast(F8)

            def mains(t):
                ps = psp.tile([128, N2], F32, tag="mm", name=f"ps{t}")
                for kk in range(KK):
                    lhsT = X1T8[:, kk, t * 256:(t + 1) * 256]
                    for (off, w) in MCHUNK:
                        rhs = X2T8[:, kk, 2 * off:2 * (off + w)] \
                            .rearrange("p (n i) -> p i n", i=2)
                        nc.tensor.matmul(ps[:, off:off + w], lhsT=lhsT, rhs=rhs,
                                         start=(kk == 0 and off % 512 == 0),
                                         stop=False, perf_mode=DR)
                for (off, w) in ACHUNK:
                    nc.tensor.matmul(ps[:, off:off + w], lhsT=ones_bf[:],
                                     rhs=row0[:, off:off + w],
                                     start=False, stop=(off == 1024))
                return ps

            def epilogue(t, ps):
                nc.vector.tensor_reduce(
                    out=gm_all[:, t, :],
                    in_=ps[:].rearrange("p (g n) -> p g n", n=GROUP),
                    axis=AX.X, op=mybir.AluOpType.max)

            for t in range(MT):
                ps = mains(t)
                epilogue(t, ps)

            # ---- sq1 via ACT square-accumulate on fp8 x1 ----
            for s in range(MT):
                nc.scalar.activation(out=workp.tile([128, D], BF, tag="scr",
                                                    name=f"scr1_{s}")[:],
                                     in_=x1p8[:, s, :], func=AF.Square,
                                     accum_out=sq1_out[:, s:s + 1])

            # ---- final per-m-tile sums of group maxima ----
            nc.vector.tensor_reduce(out=y_gm_t[:],
                                    in_=gm_all[:].rearrange("p t g -> p t g"),
                                    axis=AX.X, op=mybir.AluOpType.add)
            nc.sync.dma_start(out=y_gm.ap(), in_=y_gm_t[:])
            nc.sync.dma_start(out=y_sq1.ap(), in_=sq1_out[:])

    nc.compile()
    return nc


def get_nc():
    if "nc" not in _CACHE:
        _CACHE["nc"] = _build()
    return _CACHE["nc"]


def kernel(x1, x2):
    from concourse import bass_utils

    x1 = np.asarray(x1, dtype=np.float32)
    x2 = np.asarray(x2, dtype=np.float32)
    assert x1.shape == (B, N1, D) and x2.shape == (B, N2, D)

    nc = get_nc()
    # shard: batch b -> core b
    in_maps = [{"x1": x1[b], "x2": x2[b]} for b in range(B)]
    res = bass_utils.run_bass_kernel_spmd(nc, in_maps, core_ids=list(range(B)))

    # unshard: combine per-core partial sums (the all-reduce of the mean)
    total = 0.0
    for b in range(B):
        gm = np.asarray(res.results[b]["y_gm"], dtype=np.float64)
        sq1 = np.asarray(res.results[b]["y_sq1"], dtype=np.float64)
        total += NG * sq1.sum() - 2.0 * gm.sum()
    mean = total / (B * N1 * NG)
    return np.asarray(mean, dtype=np.float32)
